# revision 1
# baseline (speedup 1.0000x reference)
"""Mixture-of-Depths routing kernel for Trainium2 (8 NeuronCores, SPMD).

Problem (per batch row b of 4):
    logits = x[b] @ W_router.T            # [4096]
    idx    = top_k(logits, 2048)          # half the tokens
    out[b] = x[b]; out[b][idx] = x[b][idx] @ W_block.T

Sharding: 8 cores = 4 batch rows x 2 sequence halves. Each core owns 2048
tokens of one batch row. Per-core, on device:
  - router logits for the FULL row (both halves streamed token-major)
    via a fused multiply + row-reduce on VectorE,
  - the top-k threshold (= K-th largest logit) by 24 rounds of float
    bisection: count(logits >= mid) is a per-partition compare+row-reduce
    on VectorE plus a ones-matmul on TensorE that simultaneously reduces
    across partitions and broadcasts the count back to all of them,
  - transform of all 2048 own tokens (x @ W_block.T) on TensorE with the
    fp32 operands split into bf16 hi+lo pairs and three bf16 products
    (hh + hl + lh) accumulated in fp32 PSUM — ~2x the throughput of native
    fp32 matmul at a ~2^-17 relative error (the dropped ll term),
  - per-token select (transformed where logit >= threshold, else
    passthrough) with a predicated copy.

The bisection threshold is exact for this problem: the loop maintains
count(>=lo) >= K > count(>=lo+w) and narrows w to 32*2^-24 ~ 1.9e-6, far
under the ~5e-4 gap between the K-th and (K+1)-th logits, so lo lands on
exactly the K-th largest device logit and the mask selects exactly the
reference top-k set (logit values are distinct for this input
distribution; ties would make the reference itself ill-defined).
"""
import os

import numpy as np

B, S, D = 4, 4096, 1024
K_TOP = 2048
H = S // 2          # tokens per core
NT = H // 128       # 16 token tiles per core
NK = D // 128       # 8 contraction chunks
N_CORES = 8
ROUNDS = 24          # bisection of [-16,16] to ~1.9e-6, still well under
                     # the ~5e-4 gap between the K-th and (K+1)-th logits
LG_BOUND = 16.0      # |router logits| are ~N(0,1); 16 is a >10-sigma bound

_cache: dict = {}


def _build_nc():
    import concourse.bass as bass
    import concourse.mybir as mybir
    from concourse.tile import TileContext

    class _SplitWaitTC(TileContext):
        """The walrus build in this container rejects instructions carrying
        more than one sync-wait command. Tile's wait assignment routinely
        attaches several. After scheduling, move excess waits onto
        single-wait NoOps inserted before the instruction on the same
        engine (engine streams execute in order, so semantics are kept)."""

        def __exit__(self, exc_type, exc_value, traceback):
            r = super().__exit__(exc_type, exc_value, traceback)
            if exc_type is None:
                uid = 0
                for fn in self.nc.m.functions:
                    for bb in fn.blocks:
                        out = []
                        for inst in bb.instructions:
                            si = inst.sync_info
                            if si is not None and len(si.on_wait) > 1:
                                waits = list(si.on_wait)
                                si.on_wait = waits[-1:]
                                for w in waits[:-1]:
                                    uid += 1
                                    out.append(
                                        mybir.InstNoOp(
                                            name=f"I-waitsplit-{uid}",
                                            engine=inst.engine,
                                            ins=[],
                                            outs=[],
                                            sync_info=mybir.SyncInfo(
                                                on_wait=[w], on_update=[]
                                            ),
                                            text_hint="waitsplit",
                                            bass_nofuse=True,
                                        )
                                    )
                            out.append(inst)
                        bb.instructions = out
            return r

    f32 = mybir.dt.float32
    bf16 = mybir.dt.bfloat16
    u8 = mybir.dt.uint8
    ge = mybir.AluOpType.is_ge

    nc = bass.Bass("TRN2", target_bir_lowering=False, debug=False,
                   num_devices=N_CORES)
    xthi_d = nc.dram_tensor("xthi", [D, H], bf16, kind="ExternalInput")
    xtlo_d = nc.dram_tensor("xtlo", [D, H], bf16, kind="ExternalInput")
    xo_d = nc.dram_tensor("xo", [H, D], f32, kind="ExternalInput")
    xr_d = nc.dram_tensor("xr", [H, D], f32, kind="ExternalInput")
    wthi_d = nc.dram_tensor("wthi", [D, D], bf16, kind="ExternalInput")
    wtlo_d = nc.dram_tensor("wtlo", [D, D], bf16, kind="ExternalInput")
    wrb_d = nc.dram_tensor("wrb", [128, D], f32, kind="ExternalInput")
    out_d = nc.dram_tensor("out", [H, D], f32, kind="ExternalOutput")

    with _SplitWaitTC(nc) as tc:
        with (
            tc.tile_pool(name="cpool", bufs=1) as cpool,
            tc.tile_pool(name="wsp_pool", bufs=1) as wsp_pool,
            tc.tile_pool(name="xsp_pool", bufs=1) as xsp_pool,
            tc.tile_pool(name="xo_pool", bufs=6) as xo_pool,
            tc.tile_pool(name="xr_pool", bufs=6) as xr_pool,
            tc.tile_pool(name="scr_pool", bufs=2) as scr_pool,
            tc.tile_pool(name="stg_pool", bufs=12) as stg_pool,
            tc.tile_pool(name="mm_pool", bufs=3, space="PSUM") as mm_pool,
            tc.tile_pool(name="cnt_pool", bufs=2, space="PSUM") as cnt_pool,
        ):
            # ---- constants / persistent loads -------------------------
            wrb = cpool.tile([128, D], f32)
            nc.sync.dma_start(out=wrb[:], in_=wrb_d[:, :])
            ones = cpool.tile([128, 128], f32)
            nc.vector.memset(ones[:], 1.0)

            # W^T / x^T arrive pre-split from the host as bf16 hi + lo
            # pairs (x = hi + lo to ~2^-17 relative); the transform matmul
            # runs three bf16 products hh + hl + lh.
            wthi = [wsp_pool.tile([128, D], bf16, name=f"wthi{k}") for k in range(NK)]
            wtlo = [wsp_pool.tile([128, D], bf16, name=f"wtlo{k}") for k in range(NK)]
            xthi = [xsp_pool.tile([128, H], bf16, name=f"xthi{k}") for k in range(NK)]
            xtlo = [xsp_pool.tile([128, H], bf16, name=f"xtlo{k}") for k in range(NK)]
            for k in range(NK):
                ks = slice(k * 128, (k + 1) * 128)
                nc.sync.dma_start(out=wthi[k][:], in_=wthi_d[ks, :])
                nc.sync.dma_start(out=xthi[k][:], in_=xthi_d[ks, :])
                nc.sync.dma_start(out=wtlo[k][:], in_=wtlo_d[ks, :])
                nc.sync.dma_start(out=xtlo[k][:], in_=xtlo_d[ks, :])

            # ---- router logits for the full row -----------------------
            # (own half tokens streamed token-major; re-fetched later for
            # the select stage)
            lg = cpool.tile([128, 2 * NT], f32)
            for i in range(NT):
                xole = xr_pool.tile([128, D], f32, name="xole", tag="xr")
                nc.sync.dma_start(out=xole[:], in_=xo_d[i * 128:(i + 1) * 128, :])
                scr = scr_pool.tile([128, D], f32, name="scr")
                nc.vector.scalar_tensor_tensor(
                    out=scr[:], in0=xole[:], scalar=0.0, in1=wrb[:],
                    op0=mybir.AluOpType.bypass, op1=mybir.AluOpType.mult,
                    accum_out=lg[:, i:i + 1],
                )
            for j in range(NT):
                xr = xr_pool.tile([128, D], f32, name="xr", tag="xr")
                nc.sync.dma_start(out=xr[:], in_=xr_d[j * 128:(j + 1) * 128, :])
                scr = scr_pool.tile([128, D], f32, name="scr")
                nc.vector.scalar_tensor_tensor(
                    out=scr[:], in0=xr[:], scalar=0.0, in1=wrb[:],
                    op0=mybir.AluOpType.bypass, op1=mybir.AluOpType.mult,
                    accum_out=lg[:, NT + j:NT + j + 1],
                )

            # ---- threshold bisection ----------------------------------
            # state = (lo, w): interval [lo, lo+w). Each round halves w and
            # conditionally advances lo by the new w — 4 DVE ops per round,
            # all arithmetic (cond is a 0/1 float), no predicated copies.
            # With w a power of two and lo a short dyadic sum, every update
            # is exact in fp32.
            lo = cpool.tile([128, 1], f32)
            mid = cpool.tile([128, 1], f32)
            cnt = cpool.tile([128, 1], f32)
            cond = cpool.tile([128, 1], f32)
            cmpscr = cpool.tile([128, 2 * NT], f32)
            nc.vector.memset(lo[:], -LG_BOUND)
            for r in range(ROUNDS):
                wr_imm = float(2.0 * LG_BOUND * 0.5 ** (r + 1))  # interval width
                nc.vector.tensor_scalar(out=mid[:], in0=lo[:], scalar1=wr_imm,
                                        scalar2=None, op0=mybir.AluOpType.add)
                nc.vector.tensor_scalar(
                    out=cmpscr[:], in0=lg[:], scalar1=mid[:, :1], scalar2=None,
                    op0=ge, op1=mybir.AluOpType.add, accum_out=cnt[:],
                )
                cps = cnt_pool.tile([128, 1], f32, name="cps", space="PSUM")
                nc.tensor.matmul(out=cps[:], lhsT=ones[:], rhs=cnt[:],
                                 start=True, stop=True)
                nc.vector.tensor_scalar(out=cond[:], in0=cps[:],
                                        scalar1=float(K_TOP), scalar2=None, op0=ge)
                # lo += cond * w_r   (advance iff count(>=mid) >= K)
                nc.vector.scalar_tensor_tensor(
                    out=lo[:], in0=cond[:], scalar=wr_imm, in1=lo[:],
                    op0=mybir.AluOpType.mult, op1=mybir.AluOpType.add,
                )

            # ---- matmuls, stage, select, store ------------------------
            # The selects depend on the bisection threshold, which lands
            # ~100us in. To keep TensorE from throttling on PSUM-bank
            # recycling behind them, the idle Scalar engine copies each
            # accumulator to an SBUF staging tile right away (releasing
            # the bank), and the selects read the staged copy later.
            mask = cpool.tile([128, NT], u8)
            for i in range(NT):
                ts = slice(i * 128, (i + 1) * 128)
                ps0 = mm_pool.tile([128, 512], f32, name="ps0", space="PSUM")
                ps1 = mm_pool.tile([128, 512], f32, name="ps1", space="PSUM")
                for k in range(NK):
                    # hi*hi + hi*lo share one stationary load; lo*hi a second
                    nc.tensor.matmul(out=ps0[:], lhsT=xthi[k][:, ts],
                                     rhs=wthi[k][:, 0:512],
                                     start=(k == 0), stop=False)
                    nc.tensor.matmul(out=ps1[:], lhsT=xthi[k][:, ts],
                                     rhs=wthi[k][:, 512:1024],
                                     start=(k == 0), stop=False)
                    nc.tensor.matmul(out=ps0[:], lhsT=xthi[k][:, ts],
                                     rhs=wtlo[k][:, 0:512],
                                     start=False, stop=False)
                    nc.tensor.matmul(out=ps1[:], lhsT=xthi[k][:, ts],
                                     rhs=wtlo[k][:, 512:1024],
                                     start=False, stop=False)
                    nc.tensor.matmul(out=ps0[:], lhsT=xtlo[k][:, ts],
                                     rhs=wthi[k][:, 0:512],
                                     start=False, stop=(k == NK - 1))
                    nc.tensor.matmul(out=ps1[:], lhsT=xtlo[k][:, ts],
                                     rhs=wthi[k][:, 512:1024],
                                     start=False, stop=(k == NK - 1))
                stg = stg_pool.tile([128, D], f32, name="stg")
                nc.scalar.copy(out=stg[:, 0:512], in_=ps0[:])
                nc.scalar.copy(out=stg[:, 512:1024], in_=ps1[:])
                nc.vector.tensor_scalar(
                    out=mask[:, i:i + 1], in0=lg[:, i:i + 1],
                    scalar1=lo[:, :1], scalar2=None, op0=ge,
                )
                xot = xo_pool.tile([128, D], f32, name="xot")
                # defer this prefetch in the scheduler's clock so the
                # logit input streams win the DMA queues early; the
                # select below can't run before the threshold anyway
                with tc.tile_wait_until(0.08):
                    nc.sync.dma_start(out=xot[:], in_=xo_d[ts, :])
                nc.vector.copy_predicated(
                    out=xot[:],
                    mask=mask[:, i:i + 1].to_broadcast([128, D]),
                    data=stg[:],
                )
                nc.sync.dma_start(out=out_d[ts, :], in_=xot[:])
    return nc


def _get_nc():
    if "nc" not in _cache:
        _cache["nc"] = _build_nc()
    return _cache["nc"]


def _split_hi_lo(a):
    import ml_dtypes
    hi = a.astype(ml_dtypes.bfloat16)
    lo = (a - hi.astype(np.float32)).astype(ml_dtypes.bfloat16)
    return np.ascontiguousarray(hi), np.ascontiguousarray(lo)


def _make_in_maps(x, W_block, W_router):
    x = np.ascontiguousarray(np.asarray(x, dtype=np.float32))
    wt = np.ascontiguousarray(np.asarray(W_block, dtype=np.float32).T)
    wthi, wtlo = _split_hi_lo(wt)
    wr = np.asarray(W_router, dtype=np.float32).reshape(1, D)
    wrb = np.ascontiguousarray(np.broadcast_to(wr, (128, D)))
    in_maps = []
    for c in range(N_CORES):
        b, h = divmod(c, 2)
        own = x[b, h * H:(h + 1) * H, :]
        oth = x[b, (1 - h) * H:(2 - h) * H, :]
        xthi, xtlo = _split_hi_lo(np.ascontiguousarray(own.T))
        in_maps.append({
            "xthi": xthi,
            "xtlo": xtlo,
            "xo": own,
            "xr": oth,
            "wthi": wthi,
            "wtlo": wtlo,
            "wrb": wrb,
        })
    return in_maps


def run(x, W_block, W_router, trace=False):
    from concourse.bass_utils import run_bass_kernel_spmd

    nc = _get_nc()
    in_maps = _make_in_maps(x, W_block, W_router)
    res = run_bass_kernel_spmd(nc, in_maps, core_ids=list(range(N_CORES)),
                               trace=trace)
    out = np.empty((B, S, D), dtype=np.float32)
    for c in range(N_CORES):
        b, h = divmod(c, 2)
        out[b, h * H:(h + 1) * H, :] = res.results[c]["out"]
    return out, res


def kernel(x, W_block, W_router, top_k):
    assert int(top_k) == K_TOP, f"kernel compiled for top_k={K_TOP}, got {top_k}"
    trace = bool(os.environ.get("MOD_TRACE"))
    out, _ = run(x, W_block, W_router, trace=trace)
    return out



# revision 10
# speedup vs baseline: 1.1214x; 1.1214x over previous
"""Mixture-of-Depths routing kernel for Trainium2 (8 NeuronCores, SPMD).

Problem (per batch row b of 4):
    logits = x[b] @ W_router.T            # [4096]
    idx    = top_k(logits, 2048)          # half the tokens
    out[b] = x[b]; out[b][idx] = x[b][idx] @ W_block.T

Sharding: 8 cores = 4 batch rows x 2 sequence halves. Each core owns 2048
tokens of one batch row. Per-core, on device:
  - router logits for the FULL row (own half + other half streamed
    token-major) via a fused multiply + row-reduce on VectorE,
  - the top-k threshold by a 3-stage histogram search: stage 1 counts
    logits >= each of 512 compile-time-constant mids (accumulated for
    free inside the logit loop), stages 2+3 refine with 256 data-
    dependent mids each.  Counts are per-partition on VectorE; a
    ones-matmul on TensorE reduces across partitions and broadcasts.
    Final interval width 2^-20*32/512 ~ 9.5e-7, far under the ~4.5e-4
    gap between the K-th and (K+1)-th logits, so the threshold lands
    exactly on the K-th largest logit and the mask reproduces the
    reference top-k set,
  - transform of all 2048 own tokens on TensorE with a SINGLE bf16
    pass x_hi @ bf16(W_block.T - I): the rel-err of dropping the lo
    terms (~2.5e-3) is far under the 2e-2 gate, and folding -I into
    the weight makes the final select a single fused multiply-add
    out = x + mask * (x @ (W^T - I)) on VectorE, with non-selected
    rows passing through in exact fp32.
"""
import os

import numpy as np

B, S, D = 4, 4096, 1024
K_TOP = 2048
H = S // 2          # tokens per core
NT = H // 128       # 16 token tiles per core
NK = D // 128       # 8 contraction chunks
N_CORES = 8
LG_BOUND = 16.0     # |router logits| are ~N(0,1); 16 is a >10-sigma bound
NM1 = 512           # stage-1 mids (compile-time constants)
NM = 256            # stage-2/3 mids
W1 = 2.0 * LG_BOUND / NM1          # 0.0625 = 2^-4
W2 = W1 / NM                       # 2^-12
W3 = W2 / NM                       # 2^-20

_cache: dict = {}


def _build_nc():
    import concourse.bass as bass
    import concourse.mybir as mybir
    from concourse.tile import TileContext

    class _SplitWaitTC(TileContext):
        """The walrus build in this container rejects instructions carrying
        more than one sync-wait command. Tile's wait assignment routinely
        attaches several. After scheduling, move excess waits onto
        single-wait NoOps inserted before the instruction on the same
        engine (engine streams execute in order, so semantics are kept)."""

        def __exit__(self, exc_type, exc_value, traceback):
            r = super().__exit__(exc_type, exc_value, traceback)
            if exc_type is None:
                uid = 0
                for fn in self.nc.m.functions:
                    for bb in fn.blocks:
                        out = []
                        for inst in bb.instructions:
                            si = inst.sync_info
                            if si is not None and len(si.on_wait) > 1:
                                waits = list(si.on_wait)
                                si.on_wait = waits[-1:]
                                for w in waits[:-1]:
                                    uid += 1
                                    out.append(
                                        mybir.InstNoOp(
                                            name=f"I-waitsplit-{uid}",
                                            engine=inst.engine,
                                            ins=[],
                                            outs=[],
                                            sync_info=mybir.SyncInfo(
                                                on_wait=[w], on_update=[]
                                            ),
                                            text_hint="waitsplit",
                                            bass_nofuse=True,
                                        )
                                    )
                            out.append(inst)
                        bb.instructions = out
            return r

    f32 = mybir.dt.float32
    bf16 = mybir.dt.bfloat16
    ge = mybir.AluOpType.is_ge
    le = mybir.AluOpType.is_le
    mult = mybir.AluOpType.mult
    add = mybir.AluOpType.add
    bypass = mybir.AluOpType.bypass

    nc = bass.Bass("TRN2", target_bir_lowering=False, debug=False,
                   num_devices=N_CORES)
    xthi_d = nc.dram_tensor("xthi", [D, H], bf16, kind="ExternalInput")
    xo_d = nc.dram_tensor("xo", [H, D], f32, kind="ExternalInput")
    xr_d = nc.dram_tensor("xr", [H, D], f32, kind="ExternalInput")
    wthi_d = nc.dram_tensor("wthi", [D, D], bf16, kind="ExternalInput")
    wrb_d = nc.dram_tensor("wrb", [128, D], f32, kind="ExternalInput")
    out_d = nc.dram_tensor("out", [H, D], f32, kind="ExternalOutput")

    with _SplitWaitTC(nc) as tc:
        with (
            tc.tile_pool(name="cpool", bufs=1) as cpool,
            tc.tile_pool(name="wsp_pool", bufs=1) as wsp_pool,
            tc.tile_pool(name="xsp_pool", bufs=1) as xsp_pool,
            tc.tile_pool(name="xo_pool", bufs=1) as xo_pool,
            tc.tile_pool(name="xr_pool", bufs=4) as xr_pool,
            tc.tile_pool(name="scr_pool", bufs=2) as scr_pool,
            tc.tile_pool(name="stg_pool", bufs=16) as stg_pool,
            tc.tile_pool(name="mm_pool", bufs=3, space="PSUM") as mm_pool,
            tc.tile_pool(name="cnt_pool", bufs=1, space="PSUM") as cnt_pool,
        ):
            # ---- constants / persistent loads -------------------------
            wrb = cpool.tile([128, D], f32)
            nc.sync.dma_start(out=wrb[:], in_=wrb_d[:, :])
            ones = cpool.tile([128, 128], f32)
            nc.vector.memset(ones[:], 1.0)

            # j = 1..512 on every partition, as fp32 (exact: |j| << 2^24)
            jf = cpool.tile([128, NM1], f32)
            nc.gpsimd.iota(jf[:], [[1, NM1]], base=1, channel_multiplier=0,
                           allow_small_or_imprecise_dtypes=True)
            # stage-1 mids: -16 + j*W1 (compile-time constants)
            mids1 = cpool.tile([128, NM1], f32)
            nc.vector.tensor_scalar(out=mids1[:], in0=jf[:], scalar1=W1,
                                    scalar2=-LG_BOUND, op0=mult, op1=add)

            # W^T - I and x^T arrive pre-split from the host as bf16 (hi
            # only); the transform matmul is a single bf16 pass.
            wthi = [wsp_pool.tile([128, D], bf16, name=f"wthi{k}") for k in range(NK)]
            xthi = [xsp_pool.tile([128, H], bf16, name=f"xthi{k}") for k in range(NK)]
            for k in range(NK):
                ks = slice(k * 128, (k + 1) * 128)
                nc.sync.dma_start(out=wthi[k][:], in_=wthi_d[ks, :])
                nc.sync.dma_start(out=xthi[k][:], in_=xthi_d[ks, :])

            # ---- router logits + stage-1 histogram --------------------
            # Own-half tokens stay RESIDENT in SBUF (xo tiles) and feed
            # the final select; other-half tokens stream through a small
            # pool. As each logit column lands, it is also counted
            # against the 512 stage-1 mids (free: overlapped with the
            # DMA-gated logit loop).
            lg = cpool.tile([128, 2 * NT], f32)
            cnt1 = cpool.tile([128, NM1], f32)
            nc.vector.memset(cnt1[:], 0.0)
            xo = [xo_pool.tile([128, D], f32, name=f"xo{i}") for i in range(NT)]
            for i in range(NT):
                nc.sync.dma_start(out=xo[i][:], in_=xo_d[i * 128:(i + 1) * 128, :])
                scr = scr_pool.tile([128, D], f32, name="scr")
                nc.vector.scalar_tensor_tensor(
                    out=scr[:], in0=xo[i][:], scalar=0.0, in1=wrb[:],
                    op0=bypass, op1=mult,
                    accum_out=lg[:, i:i + 1],
                )
                nc.vector.scalar_tensor_tensor(
                    out=cnt1[:], in0=mids1[:], scalar=lg[:, i:i + 1], in1=cnt1[:],
                    op0=le, op1=add,
                )
            for j in range(NT):
                xr = xr_pool.tile([128, D], f32, name="xr", tag="xr")
                nc.sync.dma_start(out=xr[:], in_=xr_d[j * 128:(j + 1) * 128, :])
                scr = scr_pool.tile([128, D], f32, name="scr")
                nc.vector.scalar_tensor_tensor(
                    out=scr[:], in0=xr[:], scalar=0.0, in1=wrb[:],
                    op0=bypass, op1=mult,
                    accum_out=lg[:, NT + j:NT + j + 1],
                )
                nc.vector.scalar_tensor_tensor(
                    out=cnt1[:], in0=mids1[:], scalar=lg[:, NT + j:NT + j + 1],
                    in1=cnt1[:], op0=le, op1=add,
                )

            # ---- threshold: finish stage 1, then stages 2+3 -----------
            # Invariant: count(>= lo) >= K > count(>= lo + w). m* = the
            # number of stage mids with count >= K advances lo exactly
            # (all quantities are dyadic rationals, exact in fp32).
            lo = cpool.tile([128, 1], f32)
            mstar = cpool.tile([128, 1], f32)
            mids = cpool.tile([128, NM], f32)
            cnt = cpool.tile([128, NM], f32)
            nc.vector.memset(lo[:], -LG_BOUND)

            cps1 = cnt_pool.tile([128, NM1], f32, name="cps1", space="PSUM")
            nc.tensor.matmul(out=cps1[:], lhsT=ones[:], rhs=cnt1[:],
                             start=True, stop=True)
            nc.vector.tensor_scalar(out=cnt1[:], in0=cps1[:],
                                    scalar1=float(K_TOP), scalar2=None,
                                    op0=ge, op1=add, accum_out=mstar[:])
            nc.vector.scalar_tensor_tensor(
                out=lo[:], in0=mstar[:], scalar=W1, in1=lo[:],
                op0=mult, op1=add,
            )

            for wstage in (W2, W3):
                # mids = lo + j*w for j = 1..256
                nc.vector.tensor_scalar(out=mids[:], in0=jf[:, 0:NM],
                                        scalar1=wstage, scalar2=None, op0=mult)
                nc.vector.tensor_scalar(out=mids[:], in0=mids[:],
                                        scalar1=lo[:, 0:1], scalar2=None, op0=add)
                nc.vector.memset(cnt[:], 0.0)
                for t in range(2 * NT):
                    nc.vector.scalar_tensor_tensor(
                        out=cnt[:], in0=mids[:], scalar=lg[:, t:t + 1], in1=cnt[:],
                        op0=le, op1=add,
                    )
                cps = cnt_pool.tile([128, NM], f32, name="cps", space="PSUM")
                nc.tensor.matmul(out=cps[:], lhsT=ones[:], rhs=cnt[:],
                                 start=True, stop=True)
                nc.vector.tensor_scalar(out=cnt[:], in0=cps[:],
                                        scalar1=float(K_TOP), scalar2=None,
                                        op0=ge, op1=add, accum_out=mstar[:])
                nc.vector.scalar_tensor_tensor(
                    out=lo[:], in0=mstar[:], scalar=wstage, in1=lo[:],
                    op0=mult, op1=add,
                )

            # mask over own tokens: 1.0 where selected (f32, used as a
            # per-partition scalar multiplier in the select)
            mask = cpool.tile([128, NT], f32)
            nc.vector.tensor_scalar(out=mask[:], in0=lg[:, 0:NT],
                                    scalar1=lo[:, 0:1], scalar2=None, op0=ge)

            # ---- transform matmuls, select, store ---------------------
            # stg = x @ (W^T - I) in one bf16 pass; ScalarE drains each
            # PSUM accumulator to SBUF immediately (frees the bank);
            # select: out = x + mask * stg (exact passthrough when 0).
            for i in range(NT):
                ts = slice(i * 128, (i + 1) * 128)
                ps0 = mm_pool.tile([128, 512], f32, name="ps0", space="PSUM")
                ps1 = mm_pool.tile([128, 512], f32, name="ps1", space="PSUM")
                for k in range(NK):
                    nc.tensor.matmul(out=ps0[:], lhsT=xthi[k][:, ts],
                                     rhs=wthi[k][:, 0:512],
                                     start=(k == 0), stop=(k == NK - 1))
                    nc.tensor.matmul(out=ps1[:], lhsT=xthi[k][:, ts],
                                     rhs=wthi[k][:, 512:1024],
                                     start=(k == 0), stop=(k == NK - 1))
                # stage in bf16: halves SBUF (all 16 tiles stay resident
                # until the data-dependent threshold lands) at +2e-9-rel
                # cost on already-bf16-limited values
                stg = stg_pool.tile([128, D], bf16, name="stg")
                nc.scalar.copy(out=stg[:, 0:512], in_=ps0[:])
                nc.scalar.copy(out=stg[:, 512:1024], in_=ps1[:])
                nc.vector.scalar_tensor_tensor(
                    out=xo[i][:], in0=stg[:], scalar=mask[:, i:i + 1],
                    in1=xo[i][:], op0=mult, op1=add,
                )
                nc.sync.dma_start(out=out_d[ts, :], in_=xo[i][:])
    return nc


def _get_nc():
    if "nc" not in _cache:
        _cache["nc"] = _build_nc()
    return _cache["nc"]


def _make_in_maps(x, W_block, W_router):
    import ml_dtypes
    x = np.ascontiguousarray(np.asarray(x, dtype=np.float32))
    wt = np.asarray(W_block, dtype=np.float32).T.copy()
    wt[np.arange(D), np.arange(D)] -= 1.0        # fold -I into the weight
    wthi = np.ascontiguousarray(wt.astype(ml_dtypes.bfloat16))
    wr = np.asarray(W_router, dtype=np.float32).reshape(1, D)
    wrb = np.ascontiguousarray(np.broadcast_to(wr, (128, D)))
    in_maps = []
    for c in range(N_CORES):
        b, h = divmod(c, 2)
        own = x[b, h * H:(h + 1) * H, :]
        oth = x[b, (1 - h) * H:(2 - h) * H, :]
        xthi = np.ascontiguousarray(own.T.astype(ml_dtypes.bfloat16))
        in_maps.append({
            "xthi": xthi,
            "xo": own,
            "xr": oth,
            "wthi": wthi,
            "wrb": wrb,
        })
    return in_maps


def run(x, W_block, W_router, trace=False):
    from concourse.bass_utils import run_bass_kernel_spmd

    nc = _get_nc()
    in_maps = _make_in_maps(x, W_block, W_router)
    res = run_bass_kernel_spmd(nc, in_maps, core_ids=list(range(N_CORES)),
                               trace=trace)
    out = np.empty((B, S, D), dtype=np.float32)
    for c in range(N_CORES):
        b, h = divmod(c, 2)
        out[b, h * H:(h + 1) * H, :] = res.results[c]["out"]
    return out, res


def kernel(x, W_block, W_router, top_k):
    assert int(top_k) == K_TOP, f"kernel compiled for top_k={K_TOP}, got {top_k}"
    trace = bool(os.environ.get("MOD_TRACE"))
    out, _ = run(x, W_block, W_router, trace=trace)
    return out


# revision 15
# speedup vs baseline: 1.4645x; 1.3059x over previous
"""Mixture-of-Depths routing kernel for Trainium2 (8 NeuronCores, SPMD).

Problem (per batch row b of 4):
    logits = x[b] @ W_router.T            # [4096]
    idx    = top_k(logits, 2048)          # half the tokens
    out[b] = x[b]; out[b][idx] = x[b][idx] @ W_block.T

Sharding: 8 cores = 4 batch rows x 2 sequence halves. Each core owns 2048
tokens of one batch row. Per-core, on device:
  - router logits for the FULL row (own half + other half streamed
    token-major) via a fused multiply + row-reduce on VectorE,
  - the top-k threshold by a 3-stage histogram search: stage 1 counts
    logits >= each of 512 compile-time-constant mids (accumulated for
    free inside the logit loop), stages 2+3 refine with 256 data-
    dependent mids each.  Counts are per-partition on VectorE; a
    ones-matmul on TensorE reduces across partitions and broadcasts.
    Final interval width 2^-20*32/512 ~ 9.5e-7, far under the ~4.5e-4
    gap between the K-th and (K+1)-th logits, so the threshold lands
    exactly on the K-th largest logit and the mask reproduces the
    reference top-k set,
  - transform of all 2048 own tokens on TensorE with a SINGLE bf16
    pass x_hi @ bf16(W_block.T - I): the rel-err of dropping the lo
    terms (~2.5e-3) is far under the 2e-2 gate, and folding -I into
    the weight makes the final select a single fused multiply-add
    out = x + mask * (x @ (W^T - I)) on VectorE, with non-selected
    rows passing through in exact fp32.
"""
import os

import numpy as np

B, S, D = 4, 4096, 1024
K_TOP = 2048
H = S // 2          # tokens per core
NT = H // 128       # 16 token tiles per core
NK = D // 128       # 8 contraction chunks
N_CORES = 8
LG_BOUND = 16.0     # |router logits| are ~N(0,1); 16 is a >10-sigma bound
NM = 64             # mids per threshold stage
NS = 4              # stages: final width 32/64^4 ~ 1.9e-6 << logit gap

_cache: dict = {}


def _build_nc():
    import concourse.bass as bass
    import concourse.mybir as mybir
    from concourse.tile import TileContext

    class _SplitWaitTC(TileContext):
        """The walrus build in this container rejects instructions carrying
        more than one sync-wait command. Tile's wait assignment routinely
        attaches several. After scheduling, move excess waits onto
        single-wait NoOps inserted before the instruction on the same
        engine (engine streams execute in order, so semantics are kept)."""

        def __exit__(self, exc_type, exc_value, traceback):
            r = super().__exit__(exc_type, exc_value, traceback)
            if exc_type is None:
                uid = 0
                for fn in self.nc.m.functions:
                    for bb in fn.blocks:
                        out = []
                        for inst in bb.instructions:
                            si = inst.sync_info
                            if si is not None and len(si.on_wait) > 1:
                                waits = list(si.on_wait)
                                si.on_wait = waits[-1:]
                                for w in waits[:-1]:
                                    uid += 1
                                    out.append(
                                        mybir.InstNoOp(
                                            name=f"I-waitsplit-{uid}",
                                            engine=inst.engine,
                                            ins=[],
                                            outs=[],
                                            sync_info=mybir.SyncInfo(
                                                on_wait=[w], on_update=[]
                                            ),
                                            text_hint="waitsplit",
                                            bass_nofuse=True,
                                        )
                                    )
                            out.append(inst)
                        bb.instructions = out
            return r

    f32 = mybir.dt.float32
    bf16 = mybir.dt.bfloat16
    ge = mybir.AluOpType.is_ge
    le = mybir.AluOpType.is_le
    mult = mybir.AluOpType.mult
    add = mybir.AluOpType.add
    bypass = mybir.AluOpType.bypass

    nc = bass.Bass("TRN2", target_bir_lowering=False, debug=False,
                   num_devices=N_CORES)
    xthi_d = nc.dram_tensor("xthi", [D, H], bf16, kind="ExternalInput")
    xo_d = nc.dram_tensor("xo", [H, D], f32, kind="ExternalInput")
    xr_d = nc.dram_tensor("xr", [H, D], f32, kind="ExternalInput")
    wthi_d = nc.dram_tensor("wthi", [D, D], bf16, kind="ExternalInput")
    wrb_d = nc.dram_tensor("wrb", [128, D], f32, kind="ExternalInput")
    out_d = nc.dram_tensor("out", [H, D], f32, kind="ExternalOutput")

    with _SplitWaitTC(nc) as tc:
        with (
            tc.tile_pool(name="cpool", bufs=1) as cpool,
            tc.tile_pool(name="wsp_pool", bufs=1) as wsp_pool,
            tc.tile_pool(name="xsp_pool", bufs=1) as xsp_pool,
            tc.tile_pool(name="xo_pool", bufs=1) as xo_pool,
            tc.tile_pool(name="xr_pool", bufs=4) as xr_pool,
            tc.tile_pool(name="scr_pool", bufs=2) as scr_pool,
            tc.tile_pool(name="stg_pool", bufs=16) as stg_pool,
            tc.tile_pool(name="mm_pool", bufs=3, space="PSUM") as mm_pool,
            tc.tile_pool(name="cnt_pool", bufs=1, space="PSUM") as cnt_pool,
        ):
            # ---- constants / persistent loads -------------------------
            wrb = cpool.tile([128, D], f32)
            nc.sync.dma_start(out=wrb[:], in_=wrb_d[:, :])
            ones = cpool.tile([128, 128], f32)
            nc.vector.memset(ones[:], 1.0)

            # j = 1..NM on every partition, as fp32 (exact: |j| << 2^24)
            jf = cpool.tile([128, NM], f32)
            nc.gpsimd.iota(jf[:], [[1, NM]], base=1, channel_multiplier=0,
                           allow_small_or_imprecise_dtypes=True)

            # W^T - I and x^T arrive pre-split from the host as bf16 (hi
            # only); the transform matmul is a single bf16 pass.
            wthi = [wsp_pool.tile([128, D], bf16, name=f"wthi{k}") for k in range(NK)]
            xthi = [xsp_pool.tile([128, H], bf16, name=f"xthi{k}") for k in range(NK)]
            for k in range(NK):
                ks = slice(k * 128, (k + 1) * 128)
                nc.sync.dma_start(out=wthi[k][:], in_=wthi_d[ks, :])
                nc.sync.dma_start(out=xthi[k][:], in_=xthi_d[ks, :])

            # ---- router logits ----------------------------------------
            # Own-half tokens stay RESIDENT in SBUF (xo tiles) and feed
            # the final select; other-half tokens stream through a small
            # pool.
            lg = cpool.tile([128, 2 * NT], f32)
            xo = [xo_pool.tile([128, D], f32, name=f"xo{i}") for i in range(NT)]
            for i in range(NT):
                nc.sync.dma_start(out=xo[i][:], in_=xo_d[i * 128:(i + 1) * 128, :])
                scr = scr_pool.tile([128, D], f32, name="scr")
                nc.vector.scalar_tensor_tensor(
                    out=scr[:], in0=xo[i][:], scalar=0.0, in1=wrb[:],
                    op0=bypass, op1=mult,
                    accum_out=lg[:, i:i + 1],
                )
            for j in range(NT):
                xr = xr_pool.tile([128, D], f32, name="xr", tag="xr")
                nc.sync.dma_start(out=xr[:], in_=xr_d[j * 128:(j + 1) * 128, :])
                scr = scr_pool.tile([128, D], f32, name="scr")
                nc.vector.scalar_tensor_tensor(
                    out=scr[:], in0=xr[:], scalar=0.0, in1=wrb[:],
                    op0=bypass, op1=mult,
                    accum_out=lg[:, NT + j:NT + j + 1],
                )

            # ---- threshold: NS stages of NM mids ----------------------
            # Invariant: count(>= lo) >= K > count(>= lo + w). m* = the
            # number of stage mids with count >= K advances lo exactly
            # (all quantities are dyadic rationals, exact in fp32).
            # Per stage, ONE wide broadcast-compare builds the indicator
            # cube C[p, m, t] = (mids[p,m] <= lg[p,t]) and ONE
            # tensor_reduce sums out the token axis -- DVE time scales
            # with elements, not 32 ops per stage.
            lo = cpool.tile([128, 1], f32)
            mstar = cpool.tile([128, 1], f32)
            mids = cpool.tile([128, NM], f32)
            cnt = cpool.tile([128, NM], f32)
            cube = cpool.tile([128, NM, 2 * NT], bf16)
            nc.vector.memset(lo[:], -LG_BOUND)

            for s in range(NS):
                wstage = float(2.0 * LG_BOUND / NM ** (s + 1))
                if s == 0:
                    # mids = -16 + j*w (compile-time constants)
                    nc.vector.tensor_scalar(out=mids[:], in0=jf[:],
                                            scalar1=wstage, scalar2=-LG_BOUND,
                                            op0=mult, op1=add)
                else:
                    nc.vector.tensor_scalar(out=mids[:], in0=jf[:],
                                            scalar1=wstage, scalar2=None, op0=mult)
                    nc.vector.tensor_scalar(out=mids[:], in0=mids[:],
                                            scalar1=lo[:, 0:1], scalar2=None,
                                            op0=add)
                nc.vector.scalar_tensor_tensor(
                    out=cube[:],
                    in0=mids[:].unsqueeze(2).to_broadcast([128, NM, 2 * NT]),
                    scalar=0.0,
                    in1=lg[:].unsqueeze(1).to_broadcast([128, NM, 2 * NT]),
                    op0=bypass, op1=le,
                )
                nc.vector.tensor_reduce(out=cnt[:], in_=cube[:],
                                        axis=mybir.AxisListType.X, op=add)
                cps = cnt_pool.tile([128, NM], f32, name="cps", space="PSUM")
                nc.tensor.matmul(out=cps[:], lhsT=ones[:], rhs=cnt[:],
                                 start=True, stop=True)
                nc.vector.tensor_scalar(out=cnt[:], in0=cps[:],
                                        scalar1=float(K_TOP), scalar2=None,
                                        op0=ge, op1=add, accum_out=mstar[:])
                nc.vector.scalar_tensor_tensor(
                    out=lo[:], in0=mstar[:], scalar=wstage, in1=lo[:],
                    op0=mult, op1=add,
                )

            # mask over own tokens: 1.0 where selected (f32, used as a
            # per-partition scalar multiplier in the select)
            mask = cpool.tile([128, NT], f32)
            nc.vector.tensor_scalar(out=mask[:], in0=lg[:, 0:NT],
                                    scalar1=lo[:, 0:1], scalar2=None, op0=ge)

            # ---- transform matmuls, select, store ---------------------
            # stg = x @ (W^T - I) in one bf16 pass; ScalarE drains each
            # PSUM accumulator to SBUF immediately (frees the bank);
            # select: out = x + mask * stg (exact passthrough when 0).
            for i in range(NT):
                ts = slice(i * 128, (i + 1) * 128)
                ps0 = mm_pool.tile([128, 512], f32, name="ps0", space="PSUM")
                ps1 = mm_pool.tile([128, 512], f32, name="ps1", space="PSUM")
                for k in range(NK):
                    nc.tensor.matmul(out=ps0[:], lhsT=xthi[k][:, ts],
                                     rhs=wthi[k][:, 0:512],
                                     start=(k == 0), stop=(k == NK - 1))
                    nc.tensor.matmul(out=ps1[:], lhsT=xthi[k][:, ts],
                                     rhs=wthi[k][:, 512:1024],
                                     start=(k == 0), stop=(k == NK - 1))
                # stage in bf16: halves SBUF (all 16 tiles stay resident
                # until the data-dependent threshold lands) at +2e-9-rel
                # cost on already-bf16-limited values
                stg = stg_pool.tile([128, D], bf16, name="stg")
                nc.scalar.copy(out=stg[:, 0:512], in_=ps0[:])
                nc.scalar.copy(out=stg[:, 512:1024], in_=ps1[:])
                nc.vector.scalar_tensor_tensor(
                    out=xo[i][:], in0=stg[:], scalar=mask[:, i:i + 1],
                    in1=xo[i][:], op0=mult, op1=add,
                )
                nc.sync.dma_start(out=out_d[ts, :], in_=xo[i][:])
    return nc


def _get_nc():
    if "nc" not in _cache:
        _cache["nc"] = _build_nc()
    return _cache["nc"]


def _make_in_maps(x, W_block, W_router):
    import ml_dtypes
    x = np.ascontiguousarray(np.asarray(x, dtype=np.float32))
    wt = np.asarray(W_block, dtype=np.float32).T.copy()
    wt[np.arange(D), np.arange(D)] -= 1.0        # fold -I into the weight
    wthi = np.ascontiguousarray(wt.astype(ml_dtypes.bfloat16))
    wr = np.asarray(W_router, dtype=np.float32).reshape(1, D)
    wrb = np.ascontiguousarray(np.broadcast_to(wr, (128, D)))
    in_maps = []
    for c in range(N_CORES):
        b, h = divmod(c, 2)
        own = x[b, h * H:(h + 1) * H, :]
        oth = x[b, (1 - h) * H:(2 - h) * H, :]
        xthi = np.ascontiguousarray(own.T.astype(ml_dtypes.bfloat16))
        in_maps.append({
            "xthi": xthi,
            "xo": own,
            "xr": oth,
            "wthi": wthi,
            "wrb": wrb,
        })
    return in_maps


def run(x, W_block, W_router, trace=False):
    from concourse.bass_utils import run_bass_kernel_spmd

    nc = _get_nc()
    in_maps = _make_in_maps(x, W_block, W_router)
    res = run_bass_kernel_spmd(nc, in_maps, core_ids=list(range(N_CORES)),
                               trace=trace)
    out = np.empty((B, S, D), dtype=np.float32)
    for c in range(N_CORES):
        b, h = divmod(c, 2)
        out[b, h * H:(h + 1) * H, :] = res.results[c]["out"]
    return out, res


def kernel(x, W_block, W_router, top_k):
    assert int(top_k) == K_TOP, f"kernel compiled for top_k={K_TOP}, got {top_k}"
    trace = bool(os.environ.get("MOD_TRACE"))
    out, _ = run(x, W_block, W_router, trace=trace)
    return out
